# revision 50
# baseline (speedup 1.0000x reference)
"""Distributed 2-layer GCN (EADGNN, N=50000 E=800000 D=128) on 8 TRN2
NeuronCores via Bass/Tile.

Reference math (per layer l):
    h  = relu(A @ x @ W1[l] + b1[l])
    x' = A @ (h @ W2[l]) + b2[l]
with A = D^-1/2 (Adj + I) D^-1/2 (PyG gcn_norm, self-loops added).
b1 must be zero (the kernel commutes the target-side degree scale through
the relu); b2 is handled generally, with a fast path when it is zero.

Kernel strategy (v2):
  * Propagation commutes with the dense matmuls: A @ (x W) == (A x) W, so all
    gather/scatter happens at width D=128 instead of 4D=512.
  * A is factored: gather tables store x~ = dinv * x (source-side scale), the
    scatter is a pure 0/1 one-hot matmul, and the target-side dinv is applied
    in the epilogue (commuted through the bias-free relu for p1 stages).
  * Nodes are packed into (core, tile-of-128, slot) positions by a 2D greedy
    balancer so that every tile's edge load fits CA=8 chunks from the A half
    and CB=8 chunks from the B half (caps 1024/1024 vs. the 1000 average);
    TPC=50 tiles per core, NPAD=51200, HALF=25600 (int16 gather indices).
  * Stage 1's gather input is the raw (dinv-scaled) x — a pure kernel input —
    so the host pre-lays the per-edge source rows as a transposed edge
    stream that the kernel reads with full-bandwidth contiguous DMA instead
    of per-row gather descriptors (2x fewer effective bytes per edge).
  * Self-loop rows never touch DRAM after stage 1: each stage writes its
    per-tile output slice into a persistent SBUF table that the next stage's
    self matmul (identity one-hot) reads directly.
  * Per 128-edge chunk: one-hot S[e, t] = (iota == off_e) built on the vector
    engine (f16, 4x DVE mode), PE matmul scatter-adds into PSUM (f32).
  * Between the four propagate stages the per-core slices are AllGathered
    into replicated tables (3 collectives; the final stage output stays
    local and the host undoes the node permutation).
"""
import os
import sys

sys.path.insert(0, "/opt/trn_rl_repo")
# A previously crashed session can leave cores wedged; always reset at init.
os.environ.setdefault("NEURON_RT_RESET_CORES", "1")

import numpy as np

from concourse import bacc, mybir, tile
from concourse import bass_utils
from concourse.masks import make_identity

P = 128

REAL_CFG = dict(N=50000, D=128, L=2, NCORES=8, TPC=50, CA=8, CB=8, GBLK=5)


def derived(cfg):
    d = dict(cfg)
    d["TGT"] = cfg["TPC"] * P                 # targets per core
    d["NPAD"] = cfg["NCORES"] * d["TGT"]      # padded node count
    d["HALF"] = d["NPAD"] // 2                # rows per gather table half
    assert d["HALF"] <= 32768                 # dma_gather int16 index limit
    assert cfg["TPC"] % cfg["GBLK"] == 0
    return d


# ----------------------------------------------------------------------------
# host-side graph preprocessing
# ----------------------------------------------------------------------------

class PackingError(RuntimeError):
    pass


def preprocess(edge_index, cfg, seed=0):
    """Assign nodes to (core, tile, slot) positions and build the per-core
    gather streams (wrapped int16 indices + per-chunk target offsets)."""
    c = derived(cfg)
    N, TPC, CA, CB, NC = c["N"], c["TPC"], c["CA"], c["CB"], c["NCORES"]
    TGT, HALF = c["TGT"], c["HALF"]
    row = np.asarray(edge_index[0], np.int64)
    col = np.asarray(edge_index[1], np.int64)

    deg = np.bincount(col, minlength=N).astype(np.float64) + 1.0  # + self loop
    dinv = (1.0 / np.sqrt(deg)).astype(np.float32)

    # Split nodes into half A (cores 0..NC/2-1) and half B, balancing
    # out-degree sums (a node's half decides which gather table its
    # out-edges hit).
    outdeg = np.bincount(row, minlength=N)
    order = np.argsort(-outdeg, kind="stable")
    halfmark = np.zeros(N, bool)
    halfmark[order[::2]] = True   # True -> half A

    a_edge = halfmark[row]
    a_in = np.bincount(col[a_edge], minlength=N)
    b_in = np.bincount(col[~a_edge], minlength=N)

    ntiles_half = (NC // 2) * TPC
    capA, capB = CA * P, CB * P

    def pack_half(nodes):
        """Greedy 2D bin packing: nodes (desc by total in-degree) into
        ntiles_half tiles of <=P slots with per-tile A/B loads under caps."""
        w = a_in[nodes] + b_in[nodes]
        nodes = nodes[np.argsort(-w, kind="stable")]
        na = a_in[nodes].astype(np.int64)
        nb = b_in[nodes].astype(np.int64)
        a_load = np.zeros(ntiles_half, np.int64)
        b_load = np.zeros(ntiles_half, np.int64)
        cnt = np.zeros(ntiles_half, np.int64)
        grid = np.full((ntiles_half, P), -1, np.int64)
        full_score = np.int64(1) << 40
        for i in range(len(nodes)):
            score = np.maximum(a_load + na[i], b_load + nb[i]) \
                + (a_load + b_load) // 64
            score[cnt >= P] = full_score
            t = int(np.argmin(score))
            grid[t, cnt[t]] = nodes[i]
            a_load[t] += na[i]
            b_load[t] += nb[i]
            cnt[t] += 1
        if a_load.max() > capA or b_load.max() > capB:
            raise PackingError(
                f"tile caps exceeded: {a_load.max()}/{capA} {b_load.max()}/{capB}")
        return grid

    gridA = pack_half(np.flatnonzero(halfmark))
    gridB = pack_half(np.flatnonzero(~halfmark))
    grid = np.concatenate([gridA, gridB], 0).reshape(NC, TPC, P)

    pos = np.full(N, -1, np.int64)
    flat = grid.reshape(-1)
    valid = flat >= 0
    pos[flat[valid]] = np.flatnonzero(valid)
    assert (pos >= 0).all()

    spos, tpos = pos[row], pos[col]
    tcore = tpos // TGT
    tblk = (tpos % TGT) // P
    toff = tpos % P
    is_a = spos < HALF

    idx_full, off_full, idx_w, off_arr = {}, {}, {}, {}
    for half, CX in (("A", CA), ("B", CB)):
        sel = is_a if half == "A" else ~is_a
        sp = spos[sel] - (0 if half == "A" else HALF)
        key = tcore[sel] * TPC + tblk[sel]
        o = np.argsort(key, kind="stable")
        key_s, sp_s, to_s = key[o], sp[o], toff[sel][o]
        nblocks = NC * TPC
        cnts = np.bincount(key_s, minlength=nblocks)
        starts = np.concatenate([[0], np.cumsum(cnts)[:-1]])
        rank = np.arange(len(key_s)) - starts[key_s]
        assert rank.max(initial=0) < CX * P
        idxf = np.zeros((NC, TPC, CX * P), np.int64)
        offf = np.full((NC, TPC, CX * P), -1.0, np.float32)
        ci, bi = key_s // TPC, key_s % TPC
        idxf[ci, bi, rank] = sp_s
        offf[ci, bi, rank] = to_s
        idx_full[half], off_full[half] = idxf, offf
        # idx stream: flatten (blk, chunk, e) then wrap 16-way per dma_gather
        flat_i = idxf.reshape(NC, TPC * CX * P)
        w = flat_i.reshape(NC, -1, 16).transpose(0, 2, 1).astype(np.int16)
        idx_w[half] = np.tile(w, (1, P // 16, 1))           # [NC, 128, cols]
        off_arr[half] = offf.reshape(NC, TPC * CX, P).transpose(0, 2, 1).copy()

    dl = np.where(grid >= 0, dinv[np.maximum(grid, 0)], 0.0)  # [NC, TPC, P]
    dl = dl.transpose(0, 2, 1).astype(np.float32).copy()      # [NC, 128, TPC]

    # int16 (off, dummy-spill-col) pairs for the Pool local_scatter one-hots
    offi = {}
    for half in ("A", "B"):
        oi = off_arr[half].astype(np.int16)              # [NC, 128, TPC*CX]
        pairs = np.stack([oi, np.full_like(oi, P)], -1)  # [NC, 128, TPC*CX, 2]
        offi[half] = pairs.reshape(oi.shape[0], P, -1).copy()

    return dict(pos=pos, dinv=dinv,
                idxA=idx_w["A"], idxB=idx_w["B"],
                offA=off_arr["A"], offB=off_arr["B"],
                offAi=offi["A"], offBi=offi["B"],
                idx_fullA=idx_full["A"], idx_fullB=idx_full["B"],
                off_fullA=off_full["A"], off_fullB=off_full["B"],
                dloc=dl, d2loc=(dl * dl).copy())


# ----------------------------------------------------------------------------
# bass kernel
# ----------------------------------------------------------------------------

def build_nc(cfg, repeat=1, sim_mode=False, bzero=True):
    c = derived(cfg)
    D, L, NC, TPC, CA, CB, GBLK = (c["D"], c["L"], c["NCORES"], c["TPC"],
                                   c["CA"], c["CB"], c["GBLK"])
    TGT, NPAD, HALF = c["TGT"], c["NPAD"], c["HALF"]
    NG = TPC // GBLK
    CE = CA + CB
    f16, f32 = mybir.dt.float16, mybir.dt.float32
    f8 = mybir.dt.float8e4
    i16, i32 = mybir.dt.int16, mybir.dt.int32

    nc = bacc.Bacc("TRN2", target_bir_lowering=False, debug=False,
                   num_devices=1 if sim_mode else NC)

    def inp(name, shape, dt):
        return nc.dram_tensor(name, list(shape), dt, kind="ExternalInput").ap()

    streamA = inp("streamA", (P, TPC * CA, D), f8)
    streamB = inp("streamB", (P, TPC * CB, D), f8)
    xselfT = inp("xselfT", (P, TPC, D), f16)
    idxA = inp("idxA", (P, TPC * CA * 8), i16)
    idxB = inp("idxB", (P, TPC * CB * 8), i16)
    offA = inp("offA", (P, TPC * CA), f32)
    offB = inp("offB", (P, TPC * CB), f32)
    offAi = inp("offAi", (P, TPC * CA * 2), i16)
    offBi = inp("offBi", (P, TPC * CB * 2), i16)
    w1 = inp("w1", (L, D, 4 * D), f16)
    w2 = inp("w2", (L, 4 * D, D), f16)
    b2r = inp("b2r", (L, P, D), f32)
    dloc = inp("dloc", (P, TPC), f32)
    d2loc = inp("d2loc", (P, TPC), f32)
    y = nc.dram_tensor("y", [TGT, D], f32, kind="ExternalOutput").ap()

    rg = [list(range(NC))]

    with tile.TileContext(nc) as tc:
        with (
            tc.tile_pool(name="dram", bufs=1, space="DRAM") as dram,
            tc.tile_pool(name="const", bufs=1) as cp,
            tc.tile_pool(name="work", bufs=1) as wp,
            tc.tile_pool(name="psum", bufs=1, space="PSUM") as pp,
        ):

            iota_i = cp.tile([P, P], i32, name="iota_i")
            nc.gpsimd.iota(iota_i[:], pattern=[[1, P]], base=0, channel_multiplier=0)
            iota_f = cp.tile([P, P], f16, name="iota_f")
            nc.vector.tensor_copy(out=iota_f[:], in_=iota_i[:])
            ident = cp.tile([P, P], f16, name="ident")
            make_identity(nc, ident[:])

            w1_sb = cp.tile([P, L * 4 * D], f16, name="w1_sb")
            for l in range(L):
                nc.scalar.dma_start(out=w1_sb[:, l * 4 * D:(l + 1) * 4 * D], in_=w1[l])
            w2_sb, b2_sb = [], []
            for l in range(L):
                w2_sb.append([])
                for ci in range(4):
                    t = cp.tile([P, D], f16, name=f"w2_sb_{l}_{ci}")
                    nc.scalar.dma_start(out=t[:], in_=w2[l, ci * P:(ci + 1) * P, :])
                    w2_sb[l].append(t)
                if not bzero:
                    t = cp.tile([P, D], f32, name=f"b2_sb_{l}")
                    nc.scalar.dma_start(out=t[:], in_=b2r[l])
                    b2_sb.append(t)
            dl_sb = cp.tile([P, TPC], f32, name="dl_sb")
            nc.scalar.dma_start(out=dl_sb[:], in_=dloc[:])
            d2_sb = cp.tile([P, TPC], f32, name="d2_sb")
            nc.scalar.dma_start(out=d2_sb[:], in_=d2loc[:])

            idxA_sb = cp.tile([P, TPC * CA * 8], i16, name="idxA_sb")
            nc.gpsimd.dma_start(out=idxA_sb[:], in_=idxA[:])
            idxB_sb = cp.tile([P, TPC * CB * 8], i16, name="idxB_sb")
            nc.gpsimd.dma_start(out=idxB_sb[:], in_=idxB[:])
            offA_sb = cp.tile([P, TPC * CA], f32, name="offA_sb")
            nc.sync.dma_start(out=offA_sb[:], in_=offA[:])
            offB_sb = cp.tile([P, TPC * CB], f32, name="offB_sb")
            nc.sync.dma_start(out=offB_sb[:], in_=offB[:])
            offAi_sb = cp.tile([P, TPC * CA * 2], i16, name="offAi_sb")
            nc.scalar.dma_start(out=offAi_sb[:], in_=offAi[:])
            offBi_sb = cp.tile([P, TPC * CB * 2], i16, name="offBi_sb")
            nc.scalar.dma_start(out=offBi_sb[:], in_=offBi[:])
            ones2 = cp.tile([P, 2], f16, name="ones2")
            nc.vector.memset(ones2[:], 1.0)

            # stage-1 self rows (dinv-scaled x at this core's positions)
            xs_in = cp.tile([P, TPC, D], f16, name="xs_in")
            nc.scalar.dma_start(out=xs_in[:], in_=xselfT[:])

            rep_cell = [0]

            def stage(l, kind, table_ap, self_tile, out_loc_ap,
                      selfout_tile, final=False):
                """One propagate stage over all TPC tiles.

                kind 'p1': transposed acc [feat, tgt] + dense W1/relu/W2 ->
                t~ slice.  kind 'p2': natural acc [tgt, feat] + dinv/bias
                epilogue.  table_ap None => stage 1 (pre-gathered streams).
                self_tile: SBUF [P, TPC, D] holding this stage's own rows.
                selfout_tile: SBUF [P, TPC, D] to fill with this stage's
                output (next stage's self rows), or None when final.
                """
                rep_cell[0] += 1
                uniq = f"{kind}r{rep_cell[0]}"
                BLK = GBLK
                NGS = TPC // BLK
                if table_ap is not None:
                    tabA = table_ap[0:HALF, :]
                    tabB = table_ap[HALF:NPAD, :]
                def flush(gf):
                    # batched, lagged output write for group gf: waits are
                    # already satisfied when this issues, so it never
                    # head-of-line blocks a queue
                    lo, hi = gf * BLK * P, (gf + 1) * BLK * P
                    dst = out_loc_ap[lo:hi, :].rearrange("(b p) d -> p b d", p=P)
                    if final:
                        nc.sync.dma_start(out=dst, in_=fin_tiles[gf][:])
                    else:
                        nc.sync.dma_start(
                            out=dst,
                            in_=selfout_tile[:, gf * BLK:(gf + 1) * BLK, :])

                fin_tiles = {}
                for g in range(NGS):
                    if final:
                        fin_g = wp.tile([P, BLK, D], f32, tag="fin", bufs=3,
                                        name=f"fin_{uniq}{l}_{g}")
                        fin_tiles[g] = fin_g
                    lag = 1 if final else 2
                    if g >= lag:
                        flush(g - lag)
                    if table_ap is None:
                        # stage-1 edge stream is fp8 (host-quantized input):
                        # halves the stage-1 DMA bytes; bulk copies have no
                        # 256B elem-granularity restriction
                        gatA = wp.tile([P, BLK * CA, D], f8, tag="strA", bufs=3,
                                       name=f"strA_{uniq}{l}_{g}")
                        gatB = wp.tile([P, BLK * CB, D], f8, tag="strB", bufs=3,
                                       name=f"strB_{uniq}{l}_{g}")
                        nc.sync.dma_start(
                            out=gatA[:],
                            in_=streamA[:, g * BLK * CA:(g + 1) * BLK * CA, :])
                        nc.sync.dma_start(
                            out=gatB[:],
                            in_=streamB[:, g * BLK * CB:(g + 1) * BLK * CB, :])
                    else:
                        gatA = wp.tile([P, BLK * CA, D], f16, tag="gatA", bufs=4,
                                       name=f"gatA_{uniq}{l}_{g}")
                        gatB = wp.tile([P, BLK * CB, D], f16, tag="gatB", bufs=4,
                                       name=f"gatB_{uniq}{l}_{g}")
                        nc.gpsimd.dma_gather(
                            out_ap=gatA[:], in_ap=tabA,
                            idxs_ap=idxA_sb[:, g * BLK * CA * 8:(g + 1) * BLK * CA * 8],
                            num_idxs=BLK * CA * P, num_idxs_reg=BLK * CA * P,
                            elem_size=D, single_packet=False)
                        nc.gpsimd.dma_gather(
                            out_ap=gatB[:], in_ap=tabB,
                            idxs_ap=idxB_sb[:, g * BLK * CB * 8:(g + 1) * BLK * CB * 8],
                            num_idxs=BLK * CB * P, num_idxs_reg=BLK * CB * P,
                            elem_size=D, single_packet=False)
                    for bb in range(BLK):
                        b = g * BLK + bb
                        selfT = self_tile[:, b, :]
                        acc = pp.tile([P, D], f32, tag="acc", bufs=4,
                                      name=f"acc_{uniq}{l}_{b}", space="PSUM")
                        if kind == "p1":
                            nc.tensor.matmul(acc[:], lhsT=selfT, rhs=ident[:],
                                             start=True, stop=False)
                        else:
                            nc.tensor.matmul(acc[:], lhsT=ident[:], rhs=selfT,
                                             start=True, stop=False)
                        for j in range(CE):
                            if j < CA:
                                m_ap = gatA[:, bb * CA + j, :]
                                off_ap = offA_sb[:, b * CA + j:b * CA + j + 1]
                                offi_ap = offAi_sb[:, (b * CA + j) * 2:(b * CA + j) * 2 + 2]
                            else:
                                jj = j - CA
                                m_ap = gatB[:, bb * CB + jj, :]
                                off_ap = offB_sb[:, b * CB + jj:b * CB + jj + 1]
                                offi_ap = offBi_sb[:, (b * CB + jj) * 2:(b * CB + jj) * 2 + 2]
                            s_t = wp.tile([P, P + 2], f16, tag="s_t", bufs=24,
                                          name=f"s_{uniq}{l}_{b}_{j}")
                            # Offload some one-hot builds to the Pool engine
                            # (as local_scatter: dst[:]=0; dst[e, off_e]=1,
                            # negative pad indices skipped; col P is a dummy
                            # spill for the mandatory second index): heavily
                            # in stage 1 (no gather desc-gen there), and in
                            # tail groups of gather stages where the DMA
                            # shadow has ended and DVE would pace.
                            if table_ap is None:
                                use_pool = j in (1, 7, 10, 13)
                            else:
                                use_pool = j in (1, 9)
                            if use_pool:
                                nc.gpsimd.local_scatter(
                                    out_ap=s_t[:], data_ap=ones2[:],
                                    idxs_ap=offi_ap, channels=P,
                                    num_elems=P + 2, num_idxs=2)
                            else:
                                nc.vector.tensor_scalar(
                                    out=s_t[:, 0:P], in0=iota_f[:],
                                    scalar1=off_ap, scalar2=None,
                                    op0=mybir.AluOpType.is_equal)
                            last = j == CE - 1
                            if kind == "p1":
                                nc.tensor.matmul(acc[:], lhsT=m_ap,
                                                 rhs=s_t[:, 0:P],
                                                 start=False, stop=last)
                            else:
                                nc.tensor.matmul(acc[:], lhsT=s_t[:, 0:P],
                                                 rhs=m_ap,
                                                 start=False, stop=last)
                        if kind == "p1":
                            p1t = wp.tile([P, P], f16, tag="p1t", bufs=6,
                                          name=f"p1t_{uniq}{l}_{b}")
                            nc.scalar.activation(
                                out=p1t[:], in_=acc[:],
                                func=mybir.ActivationFunctionType.Copy,
                                bias=0.0, scale=1.0)
                            tps = pp.tile([P, D], f32, tag="tps", bufs=2,
                                          name=f"tps_{uniq}{l}_{b}", space="PSUM")
                            # all four W1 chunks land in one PSUM bank so a
                            # single wide relu covers them (each matmul
                            # overwrites its own 512B column range)
                            hps = pp.tile([P, 4, P], f32, tag="hps", bufs=2,
                                          name=f"hps_{uniq}{l}_{b}", space="PSUM")
                            for ci in range(4):
                                nc.tensor.matmul(
                                    hps[:, ci, :],
                                    lhsT=w1_sb[:, (l * 4 + ci) * P:(l * 4 + ci + 1) * P],
                                    rhs=p1t[:], start=True, stop=True)
                            hT = wp.tile([P, 4, P], f16, tag="hT", bufs=4,
                                         name=f"hT_{uniq}{l}_{b}")
                            nc.scalar.activation(
                                out=hT[:], in_=hps[:],
                                func=mybir.ActivationFunctionType.Relu,
                                bias=0.0, scale=1.0)
                            for ci in range(4):
                                nc.tensor.matmul(tps[:], lhsT=hT[:, ci, :],
                                                 rhs=w2_sb[l][ci][:],
                                                 start=(ci == 0), stop=(ci == 3))
                            nc.vector.tensor_scalar(
                                out=selfout_tile[:, b, :], in0=tps[:],
                                scalar1=d2_sb[:, b:b + 1], scalar2=None,
                                op0=mybir.AluOpType.mult)
                        else:
                            if bzero:
                                if final:
                                    nc.scalar.activation(
                                        out=fin_g[:, bb, :], in_=acc[:],
                                        func=mybir.ActivationFunctionType.Copy,
                                        bias=0.0, scale=dl_sb[:, b:b + 1])
                                else:
                                    nc.scalar.activation(
                                        out=selfout_tile[:, b, :], in_=acc[:],
                                        func=mybir.ActivationFunctionType.Copy,
                                        bias=0.0, scale=d2_sb[:, b:b + 1])
                            else:
                                tmp = wp.tile([P, D], f32, tag="ep_tmp", bufs=2,
                                              name=f"ept_{uniq}{l}_{b}")
                                nc.scalar.activation(
                                    out=tmp[:], in_=acc[:],
                                    func=mybir.ActivationFunctionType.Copy,
                                    bias=0.0, scale=dl_sb[:, b:b + 1])
                                if final:
                                    osb = wp.tile([P, D], f32, tag="osb", bufs=3,
                                                  name=f"osb_{uniq}{l}_{b}")
                                    nc.vector.tensor_tensor(
                                        out=osb[:], in0=tmp[:], in1=b2_sb[l][:],
                                        op=mybir.AluOpType.add)
                                    nc.scalar.dma_start(
                                        out=out_loc_ap[b * P:(b + 1) * P, :],
                                        in_=osb[:])
                                else:
                                    tmp2 = wp.tile([P, D], f32, tag="ep_tmp2", bufs=2,
                                                   name=f"ept2_{uniq}{l}_{b}")
                                    nc.vector.tensor_tensor(
                                        out=tmp2[:], in0=tmp[:], in1=b2_sb[l][:],
                                        op=mybir.AluOpType.add)
                                    nc.vector.tensor_scalar(
                                        out=selfout_tile[:, b, :], in0=tmp2[:],
                                        scalar1=dl_sb[:, b:b + 1], scalar2=None,
                                        op0=mybir.AluOpType.mult)

                lag = 1 if final else 2
                for gf in range(max(0, NG - lag), NG):
                    flush(gf)

            def ag(loc, tab):
                if sim_mode:
                    # TimelineSim has no collectives: stand in with the local
                    # slice copy (AG latency accounted separately)
                    nc.gpsimd.dma_start(out=tab[0:TGT, :], in_=loc[:])
                    return
                nc.gpsimd.collective_compute(
                    "AllGather", mybir.AluOpType.bypass, replica_groups=rg,
                    ins=[loc.opt()], outs=[tab.opt()])

            for _r in range(repeat):
                t_loc = dram.tile([TGT, D], f16, name=f"t_loc_{_r}")
                x1_loc = dram.tile([TGT, D], f16, name=f"x1_loc_{_r}")
                t2_loc = dram.tile([TGT, D], f16, name=f"t2_loc_{_r}")
                t_tab = dram.tile([NPAD, D], f16, name=f"t_tab_{_r}", addr_space="Shared")
                x1_tab = dram.tile([NPAD, D], f16, name=f"x1_tab_{_r}", addr_space="Shared")
                t2_tab = dram.tile([NPAD, D], f16, name=f"t2_tab_{_r}", addr_space="Shared")
                xs_t = wp.tile([P, TPC, D], f16, tag="xself", bufs=3,
                               name=f"xs_t_{_r}")
                xs_x1 = wp.tile([P, TPC, D], f16, tag="xself", bufs=3,
                                name=f"xs_x1_{_r}")
                xs_t2 = wp.tile([P, TPC, D], f16, tag="xself", bufs=3,
                                name=f"xs_t2_{_r}")
                stage(0, "p1", None, xs_in, t_loc[:], xs_t)
                ag(t_loc, t_tab)
                stage(0, "p2", t_tab[:], xs_t, x1_loc[:], xs_x1)
                ag(x1_loc, x1_tab)
                stage(1, "p1", x1_tab[:], xs_x1, t2_loc[:], xs_t2)
                ag(t2_loc, t2_tab)
                stage(1, "p2", t2_tab[:], xs_t2, y, None, final=True)

    nc.compile()
    return nc


# ----------------------------------------------------------------------------
# host glue
# ----------------------------------------------------------------------------

def make_in_maps(inputs, prep, cfg):
    c = derived(cfg)
    D, L, NC, TPC, CA, CB = (c["D"], c["L"], c["NCORES"], c["TPC"],
                             c["CA"], c["CB"])
    TGT, NPAD, HALF = c["TGT"], c["NPAD"], c["HALF"]
    x = np.asarray(inputs["x"], np.float32)
    W1 = np.asarray(inputs["W1"], np.float32)
    W2 = np.asarray(inputs["W2"], np.float32)
    b2 = np.asarray(inputs["b2"], np.float32)

    pos, dinv = prep["pos"], prep["dinv"]
    xt = np.zeros((NPAD, D), np.float16)
    xt[pos] = (x * dinv[:, None]).astype(np.float16)
    xtA, xtB = xt[:HALF], xt[HALF:]

    w1f = W1.astype(np.float16)
    w2f = W2.astype(np.float16)
    b2r = np.broadcast_to(b2[:, None, :], (L, P, D)).astype(np.float32).copy()

    from concourse import mybir as _mybir
    f8np = _mybir.dt.np(_mybir.dt.float8e4)

    def build_stream(xth, idxf, offf, CX):
        # [TPC, CX*P] positions -> [P, TPC*CX, D] fp8 edge stream
        vals = xth[idxf.reshape(-1)].reshape(TPC, CX, P, D)
        mask = (offf.reshape(TPC, CX, P) >= 0)
        vals = np.where(mask[..., None], vals, np.float16(0))
        vals = vals.astype(f8np)
        return vals.transpose(2, 0, 1, 3).reshape(P, TPC * CX, D).copy()

    in_maps = []
    for m in range(NC):
        sl = xt[m * TGT:(m + 1) * TGT].reshape(TPC, P, D)
        in_maps.append(dict(
            streamA=build_stream(xtA, prep["idx_fullA"][m],
                                 prep["off_fullA"][m], CA),
            streamB=build_stream(xtB, prep["idx_fullB"][m],
                                 prep["off_fullB"][m], CB),
            xselfT=sl.transpose(1, 0, 2).copy(),
            idxA=prep["idxA"][m], idxB=prep["idxB"][m],
            offA=prep["offA"][m], offB=prep["offB"][m],
            offAi=prep["offAi"][m], offBi=prep["offBi"][m],
            w1=w1f, w2=w2f, b2r=b2r,
            dloc=prep["dloc"][m], d2loc=prep["d2loc"][m],
        ))
    return in_maps


def assemble_output(results, prep, cfg):
    c = derived(cfg)
    D, NC, TGT = c["D"], c["NCORES"], c["TGT"]
    full = np.empty((c["NPAD"], D), np.float32)
    for m in range(NC):
        full[m * TGT:(m + 1) * TGT] = results[m]["y"]
    return full[prep["pos"]]


_NC_CACHE = {}


def get_nc(cfg, bzero=True):
    key = (cfg["CA"], cfg["CB"], bzero)
    if key not in _NC_CACHE:
        _NC_CACHE[key] = build_nc(cfg, bzero=bzero)
    return _NC_CACHE[key]


def kernel(edge_index, x, W1, b1, W2, b2, ix=0):
    cfg = REAL_CFG
    edge_index = np.asarray(edge_index, np.int64)
    inputs = dict(x=np.asarray(x), W1=np.asarray(W1), b1=np.asarray(b1),
                  W2=np.asarray(W2), b2=np.asarray(b2))
    assert edge_index.shape[0] == 2
    assert inputs["x"].shape == (cfg["N"], cfg["D"])
    # the p1 dense epilogue commutes the target-side degree scale through
    # the relu, which requires a zero b1 (true for this model family)
    assert np.all(inputs["b1"] == 0)
    bzero = bool(np.all(inputs["b2"] == 0))

    try:
        prep = preprocess(edge_index, cfg)
    except PackingError:
        # denser-than-expected tiles: retry with one spare chunk per half
        cfg = dict(cfg, CA=9, CB=9)
        prep = preprocess(edge_index, cfg)
    in_maps = make_in_maps(inputs, prep, cfg)
    nc = get_nc(cfg, bzero=bzero)
    res = bass_utils.run_bass_kernel_spmd(
        nc, in_maps, core_ids=list(range(cfg["NCORES"])), trace=False)
    return assemble_output(res.results, prep, cfg)
